# revision 1
# baseline (speedup 1.0000x reference)
"""PointWarping v3: block-winnow device kernel + exact host re-rank.

Device per core (4096 queries x 8192 points, 32 tiles of 128 queries):
augmented f32r matmul writes exact scores s = 2q.k - |k|^2 into 8 PSUM
chunks of [128,1024].  TRN2 restricts PSUM readers to ACT and
single-PSUM-operand DVE ops, and only DVE implements max, so ACT casts
6-7 chunks per tile to fp16 (alternating, to balance the two engines)
while DVE drains the rest with TT-maxes fused against early casts, then
folds the fp16 max tree down to M4[512] = per-block max over 512 blocks
of 16 columns (block b = cols == b mod 512).  DVE max/max_index emit the
top-8 block ids per query plus the block values.  Host gathers the
8x16 = 128 candidate points per query (provably a superset of the true
3-NN unless the row is flagged), re-ranks exactly in jitted jax CPU, and
computes the inverse-distance-weighted warp.  Flagged rows (duplicate
block ids from fp16 value ties, or 3rd-candidate score not strictly
above the 8th block value) are recomputed exactly on host - rare.

Execution uses a cached jit of the shard_map body (no per-call retrace)
and per-shard async device-to-host copies (single tunnel round trip).
"""

import numpy as np

B, C, N = 4, 3, 8192
NQ = 4096
NT = 32
NBLK = 512          # blocks per query row
BLKW = 16           # columns per block (stride NBLK)
EPS = 1e-10
CLAMP = 10.0

_CACHE = {}


def _build():
    if "nc" in _CACHE:
        return _CACHE["nc"]

    from contextlib import ExitStack
    from concourse import bacc, bass, tile
    from concourse import mybir

    nc = bacc.Bacc("TRN2", target_bir_lowering=False, debug=False,
                   enable_asserts=True, num_devices=1)
    f32 = mybir.dt.float32
    f32r = mybir.dt.float32r
    f16 = mybir.dt.float16
    u16 = mybir.dt.uint16
    MAX = mybir.AluOpType.max

    qaug = nc.dram_tensor("qaug", [4, NQ], f32r, kind="ExternalInput").ap()
    kaug = nc.dram_tensor("kaug", [4, N], f32r, kind="ExternalInput").ap()
    vallo = nc.dram_tensor("vallo", [128, 8 * NT], f16,
                           kind="ExternalOutput").ap()
    gidxo = nc.dram_tensor("gidxo", [128, 8 * NT], u16,
                           kind="ExternalOutput").ap()

    with tile.TileContext(nc) as tc, ExitStack() as ctx:
        cp = ctx.enter_context(tc.tile_pool(name="persist", bufs=1))
        tp = ctx.enter_context(tc.tile_pool(name="loop", bufs=4))
        pp = ctx.enter_context(tc.tile_pool(name="ps", bufs=4, space="PSUM"))

        QAUG = cp.tile([4, NQ], f32r, tag="QAUG", bufs=1, name="QAUG")
        KAUG = cp.tile([4, N], f32r, tag="KAUG", bufs=1, name="KAUG")
        # prologue: tile 0 needs QAUG[:,0:128] plus all of KAUG, so KAUG is
        # split across all three DMA queues (ACT's share lands in its idle
        # window before the first cast); remaining QAUG streams in behind.
        nc.sync.dma_start(QAUG[:, 0:128], qaug[:, 0:128])
        nc.sync.dma_start(KAUG[:, 0:1024], kaug[:, 0:1024])
        nc.sync.dma_start(KAUG[:, 1024:2730], kaug[:, 1024:2730])
        nc.scalar.dma_start(KAUG[:, 2730:5461], kaug[:, 2730:5461])
        nc.gpsimd.dma_start(KAUG[:, 5461:8192], kaug[:, 5461:8192])
        nc.sync.dma_start(QAUG[:, 128:NQ], qaug[:, 128:NQ])

        VAL8 = cp.tile([128, 8 * NT], f16, tag="VAL8", bufs=1, name="VAL8")
        GIDX8 = cp.tile([128, 8 * NT], u16, tag="GIDX8", bufs=1, name="GIDX8")

        # software-pipelined tail: M4/max8/max_index of tile t-1 are emitted
        # during tile t, interleaved between independent DVE ops so their
        # back-to-back semaphore waits are hidden by real work
        pend = None

        def tail_steps(pM, pt):
            M4 = tp.tile([128, 512], f16, tag="M4", name="M4")
            BV = VAL8[:, 8 * pt:8 * pt + 8]

            def s1():
                nc.vector.tensor_tensor(M4[:, :], pM[:, 0:512],
                                        pM[:, 512:1024], MAX)

            def s2():
                nc.vector.max(BV, M4[:, :])

            def s3():
                nc.vector.max_index(GIDX8[:, 8 * pt:8 * pt + 8], BV, M4[:, :])

            return [s1, s2, s3]

        def emit_tail(pM, pt):
            for s in tail_steps(pM, pt):
                s()

        for t in range(NT):
            lhsT = QAUG[:, bass.ts(t, 128)]
            # 8 PSUM chunks of [128, 1024]; chunk k covers cols [1024k, +1024)
            ch = []
            for k in range(8):
                P = pp.tile([128, 1024], f32, tag="P", bufs=4, name=f"P{k}")
                for i in range(2):
                    nc.tensor.matmul(
                        P[:, bass.ts(i, 512)], lhsT,
                        KAUG[:, 1024 * k + 512 * i:1024 * k + 512 * (i + 1)],
                        start=True, stop=True)
                ch.append(P)

            # drain PSUM: only ACT (casts) and single-PSUM-operand DVE maxes
            # may read it; the Pool engine supports neither PSUM nor max.
            # Alternate 6/7 ACT casts per tile to balance ACT vs DVE.
            def cast(k):
                A = tp.tile([128, 1024], f16, tag=f"A{k}", name=f"A{k}")
                nc.scalar.copy(A[:, :], ch[k][:, :])
                return A

            def tt(x, y, tag):
                Z = tp.tile([128, 1024], f16, tag=tag, name=tag)
                nc.vector.tensor_tensor(Z[:, :], x[:, :], y[:, :], MAX)
                return Z

            steps = tail_steps(*pend) if pend is not None else [
                lambda: None] * 3

            M = tp.tile([128, 1024], f16, tag="M", name="M")
            if t % 2 == 0:
                # casts c0,c1,c4,c5,c2,c3 (c4/c5 early: they gate PSUM
                # slot reuse); DVE drains c6,c7 fused with A0/A1
                A0, A1 = cast(0), cast(1)
                A4, A5 = cast(4), cast(5)
                A2, A3 = cast(2), cast(3)
                R1 = tt(ch[6], A0, "R1")
                steps[0]()
                R2 = tt(ch[7], A1, "R2")
                steps[1]()
                m1 = tt(A2, A3, "m1")
                steps[2]()
                m2 = tt(A4, A5, "m2")
                m3 = tt(R1, R2, "m3")
                m4 = tt(m1, m2, "m4")
                nc.vector.tensor_tensor(M[:, :], m3[:, :], m4[:, :], MAX)
            else:
                # casts c0,c4,c5,c6,c1,c2,c3; DVE drains c7 fused with A0
                A0 = cast(0)
                A4, A5, A6 = cast(4), cast(5), cast(6)
                A1, A2, A3 = cast(1), cast(2), cast(3)
                R1 = tt(ch[7], A0, "R1")
                steps[0]()
                m1 = tt(A1, A2, "m1")
                steps[1]()
                m2 = tt(A3, A4, "m2")
                steps[2]()
                m3 = tt(A5, A6, "m3")
                m4 = tt(m1, m2, "m4")
                m5 = tt(m3, R1, "m5")
                nc.vector.tensor_tensor(M[:, :], m4[:, :], m5[:, :], MAX)

            pend = (M, t)

        emit_tail(*pend)

        nc.sync.dma_start(vallo[:, :], VAL8[:, :])
        nc.sync.dma_start(gidxo[:, :], GIDX8[:, :])

    nc.compile()
    _CACHE["nc"] = nc
    return nc


def _get_runner():
    if "runner" in _CACHE:
        return _CACHE["runner"]

    import jax
    from jax.sharding import Mesh, PartitionSpec
    import warnings
    with warnings.catch_warnings():
        warnings.simplefilter("ignore")
        try:
            from jax.experimental.shard_map import shard_map
        except ImportError:
            from jax import shard_map
    from concourse import mybir
    from concourse.bass2jax import (
        install_neuronx_cc_hook,
        _bass_exec_p,
        partition_id_tensor,
    )

    nc = _build()
    n_cores = 8
    install_neuronx_cc_hook()
    partition_name = (nc.partition_id_tensor.name
                      if nc.partition_id_tensor else None)

    in_names, out_names, out_avals, zero_outs = [], [], [], []
    for alloc in nc.m.functions[0].allocations:
        if not isinstance(alloc, mybir.MemoryLocationSet):
            continue
        name = alloc.memorylocations[0].name
        if alloc.kind == "ExternalInput":
            if name != partition_name:
                in_names.append(name)
        elif alloc.kind == "ExternalOutput":
            out_names.append(name)
            shape = tuple(alloc.tensor_shape)
            dtype = mybir.dt.np(alloc.dtype)
            out_avals.append(jax.core.ShapedArray(shape, dtype))
            zero_outs.append((shape, dtype))
    n_params = len(in_names)
    n_outs = len(out_avals)
    all_names = list(in_names) + list(out_names)
    if partition_name is not None:
        all_names.append(partition_name)

    donate = tuple(range(n_params, n_params + n_outs))

    def _body(*args):
        operands = list(args)
        if partition_name is not None:
            operands.append(partition_id_tensor())
        outs = _bass_exec_p.bind(
            *operands,
            out_avals=tuple(out_avals),
            in_names=tuple(all_names),
            out_names=tuple(out_names),
            lowering_input_output_aliases=(),
            sim_require_finite=True,
            sim_require_nnan=True,
            nc=nc,
        )
        return tuple(outs)

    devices = jax.devices()[:n_cores]
    mesh = Mesh(np.asarray(devices), ("core",))
    in_specs = (PartitionSpec("core"),) * (n_params + n_outs)
    out_specs = (PartitionSpec("core"),) * len(out_names)
    try:
        smapped = shard_map(_body, mesh=mesh, in_specs=in_specs,
                            out_specs=out_specs, check_vma=False)
    except TypeError:
        smapped = shard_map(_body, mesh=mesh, in_specs=in_specs,
                            out_specs=out_specs, check_rep=False)
    sharded = jax.jit(smapped, donate_argnums=donate, keep_unused=True)

    runner = {
        "sharded": sharded,
        "in_names": in_names,
        "out_names": out_names,
        "zero_outs": zero_outs,
        "n_cores": n_cores,
    }
    _CACHE["runner"] = runner
    return runner


def _run_device(in_maps):
    """Run the bass kernel on 8 cores; returns per-core output dicts."""
    import jax

    r = _get_runner()
    n_cores = r["n_cores"]
    concat_in = [
        np.concatenate([np.asarray(m[name]) for m in in_maps], axis=0)
        for name in r["in_names"]
    ]
    concat_zeros = [
        np.zeros((n_cores * s[0], *s[1:]), d) for s, d in r["zero_outs"]
    ]
    out = r["sharded"](*concat_in, *concat_zeros)
    for a in out:
        for sh in a.addressable_shards:
            sh.data.copy_to_host_async()

    def collect():
        res = [np.asarray(a) for a in out]
        return [
            {name: res[i].reshape(n_cores, *r["zero_outs"][i][0])[c]
             for i, name in enumerate(r["out_names"])}
            for c in range(n_cores)
        ]

    return collect


def make_core_inputs(pos1, pos2, flow1, core):
    b, h = core // 2, core % 2
    q = pos2[b, :, h * NQ:(h + 1) * NQ]
    qaug = np.empty((4, NQ), np.float32)
    qaug[0:3] = 2.0 * q
    qaug[3] = -1.0
    k = pos1[b] + flow1[b]
    kaug = np.empty((4, N), np.float32)
    kaug[0:3] = k
    kaug[3] = (k * k).sum(axis=0)
    return {"qaug": qaug, "kaug": kaug}


def _get_combine():
    if "combine" in _CACHE:
        return _CACHE["combine"]

    import jax
    import jax.numpy as jnp

    def _one(q, kdb, flow, blk, bval):
        # q [3, NQ]; kdb/flow [N, 3]; blk [128, 8*NT] i32; bval [128, 8*NT] f16
        # The k-th NN's block is strictly outranked by at most k-1 blocks, so
        # the true 3-NN lie in the top-4 blocks (64 candidates); ties at the
        # 4/5 boundary or collapsed duplicate ids are flagged for the 128-
        # candidate host rescue.
        qc = q.reshape(3, NT, 128).transpose(2, 1, 0)        # [p, t, 3]
        blk = blk.reshape(128, NT, 8)
        blk4 = blk[..., :4]
        cand = (blk4[..., None] + NBLK * jnp.arange(BLKW, dtype=blk.dtype))
        cand = cand.reshape(128, NT, 4 * BLKW)               # [p, t, 64]
        kc = jnp.take(kdb, cand, axis=0)                     # [p, t, 64, 3]
        diff = kc - qc[:, :, None, :]
        d2 = jnp.sum(diff * diff, axis=-1)                   # [p, t, 64]
        negd3, pos3 = jax.lax.top_k(-d2, 3)
        d3 = -negd3                                          # [p, t, 3] ascending
        i3 = jnp.take_along_axis(cand, pos3, axis=-1)        # point indices
        dist = jnp.maximum(jnp.sqrt(jnp.maximum(d3, 0.0)), EPS)
        inv = 1.0 / dist
        w = inv / jnp.sum(inv, axis=-1, keepdims=True)
        f3 = jnp.take(flow, i3, axis=0)                      # [p, t, 3, 3]
        flow2 = jnp.sum(w[..., None] * f3, axis=-2)          # [p, t, 3]
        res = jnp.clip(qc - flow2, -CLAMP, CLAMP)
        out = res.transpose(2, 1, 0).reshape(3, NQ)

        sb = jnp.sort(blk4, axis=-1)
        dup = jnp.any(sb[..., 1:] == sb[..., :-1], axis=-1)  # [p, t]
        qn = jnp.sum(qc * qc, axis=-1)                       # [p, t]
        s3c = (qn - d3[..., 2]).astype(jnp.float16)
        bv = bval.reshape(128, NT, 8)
        flag = dup | (s3c <= bv[..., 3]) | (bv[..., 3] == bv[..., 4])
        return out, flag

    fn = jax.jit(jax.vmap(_one))
    _CACHE["combine"] = fn
    return fn


def _rescue_128(q, kdb, flow, blk, bval, rows):
    """128-candidate exact re-rank for rows flagged by the 64-cand pass.

    Returns (cols, fixed [m,3], still_flagged) - rows whose 8-block
    candidate set is itself suspect (old flag logic) stay flagged.
    """
    p_idx, t_idx = rows
    cols = 128 * t_idx + p_idx
    qf = q[:, cols].T                                        # [m, 3]
    b8 = blk.reshape(128, NT, 8)[p_idx, t_idx].astype(np.int64)   # [m, 8]
    cand = (b8[:, :, None] + NBLK * np.arange(BLKW)).reshape(-1, 8 * BLKW)
    kc = kdb[cand]                                           # [m, 128, 3]
    d2 = ((kc - qf[:, None, :]) ** 2).sum(-1, dtype=np.float32)
    order = np.argsort(d2, axis=1, kind="stable")[:, :3]
    d3 = np.take_along_axis(d2, order, 1)
    i3 = np.take_along_axis(cand, order, 1)
    dist = np.maximum(np.sqrt(np.maximum(d3, 0.0)), EPS)
    inv = 1.0 / dist
    w = inv / inv.sum(-1, keepdims=True)
    flow2 = (w[..., None] * flow[i3]).sum(-2, dtype=np.float32)
    fixed = np.clip(qf - flow2, -CLAMP, CLAMP)

    bv = bval.reshape(128, NT, 8)[p_idx, t_idx]              # [m, 8] f16
    sb = np.sort(b8, axis=-1)
    dup = (np.diff(sb, axis=-1) == 0).any(-1)
    s3c = ((qf * qf).sum(-1) - d3[:, 2]).astype(np.float16)
    still = dup | (s3c <= bv[:, 7])
    return cols, fixed, still


def _fallback_exact(q, kdb, flow, rows):
    """Exact 3-NN warp for flagged query rows. rows: (p_idx, t_idx)."""
    p_idx, t_idx = rows
    cols = 128 * t_idx + p_idx
    qf = q[:, cols].T                                        # [m, 3]
    d2 = ((qf[:, None, :] - kdb[None, :, :]) ** 2).sum(-1, dtype=np.float32)
    order = np.argsort(d2, axis=1, kind="stable")[:, :3]
    d3 = np.take_along_axis(d2, order, 1)
    dist = np.maximum(np.sqrt(np.maximum(d3, 0.0)), EPS)
    inv = 1.0 / dist
    w = inv / inv.sum(-1, keepdims=True)
    flow2 = (w[..., None] * flow[order]).sum(-2, dtype=np.float32)
    return cols, np.clip(qf - flow2, -CLAMP, CLAMP)          # [m, 3]


def kernel(pos1, pos2, flow1):
    import jax

    pos1 = np.asarray(pos1, dtype=np.float32)
    pos2 = np.asarray(pos2, dtype=np.float32)
    flow1 = np.asarray(flow1, dtype=np.float32)

    in_maps = [make_core_inputs(pos1, pos2, flow1, c) for c in range(8)]
    collect = _run_device(in_maps)

    # host-side array prep overlaps the device round trip
    q_all = np.stack([pos2[c // 2, :, (c % 2) * NQ:(c % 2 + 1) * NQ]
                      for c in range(8)])
    kdb_all = np.stack([
        np.ascontiguousarray(in_maps[c]["kaug"][0:3].T) for c in range(8)
    ])
    flow_all = np.stack([
        np.ascontiguousarray(flow1[c // 2].T) for c in range(8)
    ])

    outs = collect()
    blk_all = np.stack([outs[c]["gidxo"].astype(np.int32) for c in range(8)])
    bval_all = np.stack([outs[c]["vallo"] for c in range(8)])

    cpu = jax.devices("cpu")[0]
    with jax.default_device(cpu):
        res, flag = _get_combine()(q_all, kdb_all, flow_all, blk_all, bval_all)
    res = np.asarray(res)
    flag = np.asarray(flag)

    full = np.empty((B, C, N), dtype=np.float32)
    for c in range(8):
        b, h = c // 2, c % 2
        out_c = res[c]
        if flag[c].any():
            rows = np.nonzero(flag[c])
            cols, fixed, still = _rescue_128(
                q_all[c], kdb_all[c], flow_all[c],
                blk_all[c], bval_all[c], rows)
            out_c = out_c.copy()
            out_c[:, cols] = fixed.T
            if still.any():
                rows2 = (rows[0][still], rows[1][still])
                cols2, fixed2 = _fallback_exact(
                    q_all[c], kdb_all[c], flow_all[c], rows2)
                out_c[:, cols2] = fixed2.T
        full[b, :, h * NQ:(h + 1) * NQ] = out_c
    return full



# revision 6
# speedup vs baseline: 3.4215x; 3.4215x over previous
"""PointWarping v4: exact on-device 3-NN, index-only readback.

The axon tunnel has ~80ms fixed dispatch latency and slow transfers, so
the design minimizes tunnel bytes and host work:

Device per core (4096 queries x 8192 points, 32 tiles of 128 queries):
augmented f32r matmul writes exact scores s = 2q.k - |k|^2 into 8 PSUM
chunks of [128,1024] (ranking by s descending == ranking by squared
distance ascending, since |q|^2 is constant per query row).  ACT drains
each chunk to a [128,8192] f32 SBUF row; one DVE max8 gives the top-8
scores per query and one DVE max_index gives their column indices
(ties resolved to distinct positions, lowest index first — matching
jax.lax.top_k).  max_index writes [128,8] straight into an overlapping
window of the persistent index tile so cols 3t..3t+3 keep tile t's
top-3; a single [128,96] u16 DMA is the only output (24KB/core).

Host: gather the 3 neighbor coords/flows per query (numpy fancy index),
recompute exact f32 distances, inverse-distance weights, warp, clip.
No re-rank, no rescue path.  Output placeholder buffers are created
in-graph (jnp.zeros) so no zero upload crosses the tunnel.
"""

import numpy as np

B, C, N = 4, 3, 8192
NQ = 4096
NT = 32
EPS = 1e-10
CLAMP = 10.0

_CACHE = {}


def _build():
    if "nc" in _CACHE:
        return _CACHE["nc"]

    from contextlib import ExitStack
    from concourse import bacc, bass, tile
    from concourse import mybir

    nc = bacc.Bacc("TRN2", target_bir_lowering=False, debug=False,
                   enable_asserts=True, num_devices=1)
    f32 = mybir.dt.float32
    f32r = mybir.dt.float32r
    u16 = mybir.dt.uint16

    qaug = nc.dram_tensor("qaug", [4, NQ], f32, kind="ExternalInput").ap()
    kaug = nc.dram_tensor("kaug", [4, N], f32, kind="ExternalInput").ap()
    gidxo = nc.dram_tensor("gidxo", [128, 3 * NT], u16,
                           kind="ExternalOutput").ap()

    with tile.TileContext(nc) as tc, ExitStack() as ctx:
        cp = ctx.enter_context(tc.tile_pool(name="persist", bufs=1))
        tp = ctx.enter_context(tc.tile_pool(name="loop", bufs=2))
        pp = ctx.enter_context(tc.tile_pool(name="ps", bufs=4, space="PSUM"))

        QAUG = cp.tile([4, NQ], f32, tag="QAUG", bufs=1, name="QAUG")
        KAUG = cp.tile([4, N], f32, tag="KAUG", bufs=1, name="KAUG")
        # prologue: tile 0 needs QAUG[:,0:128] plus all of KAUG, so KAUG is
        # split across DMA queues; remaining QAUG streams in behind.
        nc.sync.dma_start(QAUG[:, 0:128], qaug[:, 0:128])
        nc.sync.dma_start(KAUG[:, 0:1024], kaug[:, 0:1024])
        nc.sync.dma_start(KAUG[:, 1024:2730], kaug[:, 1024:2730])
        nc.scalar.dma_start(KAUG[:, 2730:5461], kaug[:, 2730:5461])
        nc.gpsimd.dma_start(KAUG[:, 5461:8192], kaug[:, 5461:8192])
        nc.sync.dma_start(QAUG[:, 128:NQ], qaug[:, 128:NQ])

        # cols 3t..3t+3 hold tile t's top-3; max_index writes 8 cols per
        # tile, the 5 extra are overwritten by the next tile (engine-ordered
        # on DVE), with 5 pad cols for the last tile.
        GIDX3 = cp.tile([128, 3 * NT + 5], u16, tag="GIDX3", bufs=1,
                        name="GIDX3")

        for t in range(NT):
            lhsT = QAUG[:, bass.ts(t, 128)]
            S = tp.tile([128, N], f32, tag="S", name="S")
            for k in range(8):
                P = pp.tile([128, 1024], f32, tag="P", bufs=4, name=f"P{k}")
                for i in range(2):
                    nc.tensor.matmul(
                        P[:, bass.ts(i, 512)], lhsT,
                        KAUG[:, 1024 * k + 512 * i:1024 * k + 512 * (i + 1)],
                        start=True, stop=True)
                nc.scalar.copy(S[:, 1024 * k:1024 * (k + 1)], P[:, :])
            TOP = tp.tile([128, 8], f32, tag="TOP", name="TOP")
            nc.vector.max(TOP[:, :], S[:, :])
            nc.vector.max_index(GIDX3[:, 3 * t:3 * t + 8], TOP[:, :], S[:, :])

        nc.sync.dma_start(gidxo[:, :], GIDX3[:, 0:3 * NT])

    nc.compile()
    _CACHE["nc"] = nc
    return nc


def _get_runner():
    if "runner" in _CACHE:
        return _CACHE["runner"]

    import jax
    import jax.numpy as jnp
    from jax.sharding import Mesh, PartitionSpec
    import warnings
    with warnings.catch_warnings():
        warnings.simplefilter("ignore")
        try:
            from jax.experimental.shard_map import shard_map
        except ImportError:
            from jax import shard_map
    from concourse import mybir
    from concourse.bass2jax import (
        install_neuronx_cc_hook,
        _bass_exec_p,
        partition_id_tensor,
    )

    nc = _build()
    n_cores = 8
    install_neuronx_cc_hook()
    partition_name = (nc.partition_id_tensor.name
                      if nc.partition_id_tensor else None)

    in_names, out_names, out_avals = [], [], []
    for alloc in nc.m.functions[0].allocations:
        if not isinstance(alloc, mybir.MemoryLocationSet):
            continue
        name = alloc.memorylocations[0].name
        if alloc.kind == "ExternalInput":
            if name != partition_name:
                in_names.append(name)
        elif alloc.kind == "ExternalOutput":
            out_names.append(name)
            shape = tuple(alloc.tensor_shape)
            dtype = mybir.dt.np(alloc.dtype)
            out_avals.append(jax.core.ShapedArray(shape, dtype))
    all_names = list(in_names) + list(out_names)
    if partition_name is not None:
        all_names.append(partition_name)

    def _body(*args):
        operands = list(args)
        if partition_name is not None:
            operands.append(partition_id_tensor())
        outs = _bass_exec_p.bind(
            *operands,
            out_avals=tuple(out_avals),
            in_names=tuple(all_names),
            out_names=tuple(out_names),
            lowering_input_output_aliases=(),
            sim_require_finite=True,
            sim_require_nnan=True,
            nc=nc,
        )
        return tuple(outs)

    devices = jax.devices()[:n_cores]
    mesh = Mesh(np.asarray(devices), ("core",))
    in_specs = (PartitionSpec("core"),) * (len(in_names) + len(out_names))
    out_specs = (PartitionSpec("core"),) * len(out_names)
    try:
        smapped = shard_map(_body, mesh=mesh, in_specs=in_specs,
                            out_specs=out_specs, check_vma=False)
    except TypeError:
        smapped = shard_map(_body, mesh=mesh, in_specs=in_specs,
                            out_specs=out_specs, check_rep=False)
    sharded = jax.jit(smapped)

    runner = {
        "sharded": sharded,
        "in_names": in_names,
        "out_names": out_names,
        "out_shapes": [(tuple(a.shape), a.dtype) for a in out_avals],
        "n_cores": n_cores,
    }
    _CACHE["runner"] = runner
    return runner


def _run_device(in_maps):
    """Run the bass kernel on 8 cores; returns per-core output dicts."""

    r = _get_runner()
    n_cores = r["n_cores"]
    concat_in = [
        np.concatenate([np.asarray(m[name]) for m in in_maps], axis=0)
        for name in r["in_names"]
    ]
    concat_zeros = [
        np.zeros((n_cores * s[0], *s[1:]), d) for s, d in r["out_shapes"]
    ]
    out = r["sharded"](*concat_in, *concat_zeros)
    for a in out:
        for sh in a.addressable_shards:
            sh.data.copy_to_host_async()

    def collect():
        res = [np.asarray(a) for a in out]
        return [
            {name: res[i].reshape(n_cores, *r["out_shapes"][i][0])[c]
             for i, name in enumerate(r["out_names"])}
            for c in range(n_cores)
        ]

    return collect


def make_core_inputs(pos1, pos2, flow1, core):
    b, h = core // 2, core % 2
    q = pos2[b, :, h * NQ:(h + 1) * NQ]
    qaug = np.empty((4, NQ), np.float32)
    qaug[0:3] = 2.0 * q
    qaug[3] = -1.0
    k = pos1[b] + flow1[b]
    kaug = np.empty((4, N), np.float32)
    kaug[0:3] = k
    kaug[3] = (k * k).sum(axis=0)
    return {"qaug": qaug, "kaug": kaug}


def kernel(pos1, pos2, flow1):
    pos1 = np.asarray(pos1, dtype=np.float32)
    pos2 = np.asarray(pos2, dtype=np.float32)
    flow1 = np.asarray(flow1, dtype=np.float32)

    in_maps = [make_core_inputs(pos1, pos2, flow1, c) for c in range(8)]
    collect = _run_device(in_maps)

    # host-side array prep overlaps the device round trip
    q_all = [pos2[c // 2, :, (c % 2) * NQ:(c % 2 + 1) * NQ].T
             for c in range(8)]                                  # [4096, 3]
    kdb = [np.ascontiguousarray(in_maps[2 * b]["kaug"][0:3].T)
           for b in range(B)]                                    # [8192, 3]
    flow_t = [np.ascontiguousarray(flow1[b].T) for b in range(B)]

    outs = collect()

    full = np.empty((B, C, N), dtype=np.float32)
    for c in range(8):
        b, h = c // 2, c % 2
        # gidxo[p, 3t+j] = j-th NN of query 128t+p -> query-major [4096, 3]
        idx = outs[c]["gidxo"].astype(np.int64).reshape(128, NT, 3)
        idx = np.minimum(idx.transpose(1, 0, 2).reshape(NQ, 3), N - 1)
        qf = q_all[c]                                            # [4096, 3]
        g = kdb[b][idx]                                          # [4096,3,3]
        d2 = ((g - qf[:, None, :]) ** 2).sum(-1, dtype=np.float32)
        dist = np.maximum(np.sqrt(d2), EPS)
        inv = 1.0 / dist
        w = inv / inv.sum(-1, keepdims=True)                     # [4096, 3]
        fg = flow_t[b][idx]                                      # [4096,3,3]
        flow2 = (w[..., None] * fg).sum(1, dtype=np.float32)     # [4096, 3]
        res = np.clip(qf - flow2, -CLAMP, CLAMP)
        full[b, :, h * NQ:(h + 1) * NQ] = res.T
    return full


# revision 11
# speedup vs baseline: 3.5318x; 1.0322x over previous
"""PointWarping v5: exact on-device 3-NN, index-only readback.

The axon tunnel has ~80ms fixed dispatch latency and ~100MB/s effective
transfer, so the design minimizes tunnel bytes and host work:

Device per core (4096 queries x 8192 points, 32 tiles of 128 queries):
inputs are RAW coords: qr [3,4096] (queries) and kr [3,8192] (database).
A prologue builds the augmented operands on device: QAUG = [q; -1]
(memset row), KAUG = [k; |k|^2/2] (DVE square + cross-partition row
adds).  Ranking uses s = q.k - |k|^2/2 (= s_true/2 up to the constant
|q|^2 row term), computed with true-f32 matmul (f32r is ~1e-5 noisy and
misranks ~16% of rows).  8 PSUM chunks per tile are ACT-drained to a
[128,8192] f32 SBUF row; one DVE max8 + one max_index give the exact
top-8 columns per query, ties resolved to distinct positions lowest-
index-first, matching jax.lax.top_k.  max_index writes [128,8] into an
overlapping window of the persistent index tile so cols 3t..3t+3 keep
tile t's top-3; one [128,96] u16 DMA is the only output (24KB/core).

Host: one vectorized numpy pass over all 8 cores gathers the 3
neighbor coords/flows per query, recomputes exact f32 distances,
inverse-distance weights, warp, clip.  No re-rank, no rescue path.
Output placeholder buffers are uploaded once and reused across calls
(their content is never read by the device program).
"""

import numpy as np

B, C, N = 4, 3, 8192
NQ = 4096
NT = 32
EPS = 1e-10
CLAMP = 10.0

_CACHE = {}


def _build():
    if "nc" in _CACHE:
        return _CACHE["nc"]

    from contextlib import ExitStack
    from concourse import bacc, bass, tile
    from concourse import mybir

    nc = bacc.Bacc("TRN2", target_bir_lowering=False, debug=False,
                   enable_asserts=True, num_devices=1)
    f32 = mybir.dt.float32
    u16 = mybir.dt.uint16
    MULT = mybir.AluOpType.mult

    qr = nc.dram_tensor("qr", [3, NQ], f32, kind="ExternalInput").ap()
    kr = nc.dram_tensor("kr", [3, N], f32, kind="ExternalInput").ap()
    gidxo = nc.dram_tensor("gidxo", [128, 3 * NT], u16,
                           kind="ExternalOutput").ap()

    with tile.TileContext(nc) as tc, ExitStack() as ctx:
        cp = ctx.enter_context(tc.tile_pool(name="persist", bufs=1))
        tp = ctx.enter_context(tc.tile_pool(name="loop", bufs=2))
        pp = ctx.enter_context(tc.tile_pool(name="ps", bufs=4, space="PSUM"))

        # s = q.k - |k|^2/2 via one K=6 contraction:
        #   lhsT rows = [q0,q1,q2,-.5,-.5,-.5], rhs rows = [k0,k1,k2,k0^2,
        #   k1^2,k2^2].  Compute ops cannot start at partition>0, so the
        #   constant rows come from a full-tile memset overwritten by the q
        #   DMA, and the squared rows are squared at partition 0 then moved
        #   to partitions 3:6 by an SBUF->SBUF DMA (DMA may target any
        #   partition).
        Q6 = cp.tile([6, NQ], f32, tag="Q6", bufs=1, name="Q6")
        K6 = cp.tile([6, N], f32, tag="K6", bufs=1, name="K6")
        KSQ = cp.tile([3, N], f32, tag="KSQ", bufs=1, name="KSQ")

        nc.vector.memset(Q6[:, :], -0.5)
        nc.sync.dma_start(Q6[0:3, 0:128], qr[:, 0:128])
        nc.sync.dma_start(K6[0:3, 0:2730], kr[:, 0:2730])
        nc.scalar.dma_start(K6[0:3, 2730:5461], kr[:, 2730:5461])
        nc.gpsimd.dma_start(K6[0:3, 5461:8192], kr[:, 5461:8192])
        nc.sync.dma_start(Q6[0:3, 128:NQ], qr[:, 128:NQ])

        nc.vector.tensor_tensor(KSQ[:, :], K6[0:3, :], K6[0:3, :], MULT)
        nc.sync.dma_start(K6[3:6, :], KSQ[:, :])

        # cols 3t..3t+3 hold tile t's top-3; max_index writes 8 cols per
        # tile, the 5 extra are overwritten by the next tile (engine-ordered
        # on DVE), with 5 pad cols for the last tile.
        GIDX3 = cp.tile([128, 3 * NT + 5], u16, tag="GIDX3", bufs=1,
                        name="GIDX3")

        for t in range(NT):
            lhsT = Q6[:, bass.ts(t, 128)]
            S = tp.tile([128, N], f32, tag="S", name="S")
            for k in range(8):
                P = pp.tile([128, 1024], f32, tag="P", bufs=4, name=f"P{k}")
                for i in range(2):
                    nc.tensor.matmul(
                        P[:, bass.ts(i, 512)], lhsT,
                        K6[:, 1024 * k + 512 * i:1024 * k + 512 * (i + 1)],
                        start=True, stop=True)
                nc.scalar.copy(S[:, 1024 * k:1024 * (k + 1)], P[:, :])
            TOP = tp.tile([128, 8], f32, tag="TOP", name="TOP")
            nc.vector.max(TOP[:, :], S[:, :])
            nc.vector.max_index(GIDX3[:, 3 * t:3 * t + 8], TOP[:, :], S[:, :])

        nc.sync.dma_start(gidxo[:, :], GIDX3[:, 0:3 * NT])

    nc.compile()
    _CACHE["nc"] = nc
    return nc


def _get_runner():
    if "runner" in _CACHE:
        return _CACHE["runner"]

    import jax
    from jax.sharding import Mesh, PartitionSpec, NamedSharding
    import warnings
    with warnings.catch_warnings():
        warnings.simplefilter("ignore")
        try:
            from jax.experimental.shard_map import shard_map
        except ImportError:
            from jax import shard_map
    from concourse import mybir
    from concourse.bass2jax import (
        install_neuronx_cc_hook,
        _bass_exec_p,
        partition_id_tensor,
    )

    nc = _build()
    n_cores = 8
    install_neuronx_cc_hook()
    partition_name = (nc.partition_id_tensor.name
                      if nc.partition_id_tensor else None)

    in_names, out_names, out_avals = [], [], []
    for alloc in nc.m.functions[0].allocations:
        if not isinstance(alloc, mybir.MemoryLocationSet):
            continue
        name = alloc.memorylocations[0].name
        if alloc.kind == "ExternalInput":
            if name != partition_name:
                in_names.append(name)
        elif alloc.kind == "ExternalOutput":
            out_names.append(name)
            shape = tuple(alloc.tensor_shape)
            dtype = mybir.dt.np(alloc.dtype)
            out_avals.append(jax.core.ShapedArray(shape, dtype))
    all_names = list(in_names) + list(out_names)
    if partition_name is not None:
        all_names.append(partition_name)

    def _body(*args):
        operands = list(args)
        if partition_name is not None:
            operands.append(partition_id_tensor())
        outs = _bass_exec_p.bind(
            *operands,
            out_avals=tuple(out_avals),
            in_names=tuple(all_names),
            out_names=tuple(out_names),
            lowering_input_output_aliases=(),
            sim_require_finite=True,
            sim_require_nnan=True,
            nc=nc,
        )
        return tuple(outs)

    devices = jax.devices()[:n_cores]
    mesh = Mesh(np.asarray(devices), ("core",))
    in_specs = (PartitionSpec("core"),) * (len(in_names) + len(out_names))
    out_specs = (PartitionSpec("core"),) * len(out_names)
    try:
        smapped = shard_map(_body, mesh=mesh, in_specs=in_specs,
                            out_specs=out_specs, check_vma=False)
    except TypeError:
        smapped = shard_map(_body, mesh=mesh, in_specs=in_specs,
                            out_specs=out_specs, check_rep=False)
    sharded = jax.jit(smapped)

    # output placeholder buffers: uploaded once, reused every call (the
    # device program never reads them)
    sh = NamedSharding(mesh, PartitionSpec("core"))
    resident_zeros = [
        jax.device_put(
            np.zeros((n_cores * a.shape[0], *a.shape[1:]), a.dtype), sh)
        for a in out_avals
    ]
    for z in resident_zeros:
        z.block_until_ready()

    runner = {
        "sharded": sharded,
        "in_names": in_names,
        "out_names": out_names,
        "out_shapes": [(tuple(a.shape), a.dtype) for a in out_avals],
        "zeros": resident_zeros,
        "n_cores": n_cores,
    }
    _CACHE["runner"] = runner
    return runner


def _run_device(concat_in):
    """Run the bass kernel on 8 cores; returns a collect() closure."""

    r = _get_runner()
    n_cores = r["n_cores"]
    out = r["sharded"](*concat_in, *r["zeros"])
    for a in out:
        for sh in a.addressable_shards:
            sh.data.copy_to_host_async()

    def collect():
        return [np.asarray(a) for a in out]

    return collect


def kernel(pos1, pos2, flow1):
    pos1 = np.asarray(pos1, dtype=np.float32)
    pos2 = np.asarray(pos2, dtype=np.float32)
    flow1 = np.asarray(flow1, dtype=np.float32)

    # core c = 2b + h handles queries pos2[b, :, h*NQ:(h+1)*NQ] against the
    # full batch-b database k = pos1[b] + flow1[b]
    k_all = pos1 + flow1                                     # [4, 3, 8192]
    qr_in = np.concatenate(
        [pos2[c // 2, :, (c % 2) * NQ:(c % 2 + 1) * NQ] for c in range(8)],
        axis=0)                                              # [24, 4096]
    kr_in = np.concatenate([k_all[c // 2] for c in range(8)], axis=0)

    collect = _run_device([qr_in, kr_in])

    # host-side array prep overlaps the device round trip
    q_all = pos2.transpose(0, 2, 1).reshape(8, NQ, 3)        # [8, 4096, 3]
    kdb_flat = k_all.transpose(0, 2, 1).reshape(B * N, 3)    # [32768, 3]
    flow_flat = flow1.transpose(0, 2, 1).reshape(B * N, 3)
    base = (np.arange(8, dtype=np.int64) // 2 * N)[:, None, None]

    res = collect()[0]                                       # [1024, 96] u16
    # gidxo[p, 3t+j] per core -> query-major [8, 4096, 3]
    idx = res.reshape(8, 128, NT, 3).transpose(0, 2, 1, 3).reshape(8, NQ, 3)
    idx = np.minimum(idx.astype(np.int64), N - 1) + base     # flat into [32768]

    g = kdb_flat[idx]                                        # [8, 4096, 3, 3]
    d = g - q_all[:, :, None, :]
    d2 = np.einsum("qncj,qncj->qnc", d, d, dtype=np.float32)
    dist = np.maximum(np.sqrt(d2), EPS)
    inv = 1.0 / dist
    w = inv / inv.sum(-1, keepdims=True)                     # [8, 4096, 3]
    fg = flow_flat[idx]                                      # [8, 4096, 3, 3]
    flow2 = np.einsum("qnc,qncj->qnj", w, fg)                # [8, 4096, 3]
    res8 = np.clip(q_all - flow2, -CLAMP, CLAMP)             # [8, 4096, 3]
    # [8, 4096, 3] -> [4, 3, 8192] with n = h*NQ + pos
    return np.ascontiguousarray(
        res8.reshape(B, 2, NQ, 3).transpose(0, 3, 1, 2).reshape(B, C, N))


# revision 14
# speedup vs baseline: 3.8627x; 1.0937x over previous
"""PointWarping v5: exact on-device 3-NN, index-only readback.

The axon tunnel has ~80ms fixed dispatch latency and ~100MB/s effective
transfer, so the design minimizes tunnel bytes and host work:

Device per core (4096 queries x 8192 points, 32 tiles of 128 queries):
inputs are RAW coords: qr [3,4096] (queries) and kr [3,8192] (database).
A prologue builds the augmented operands on device: QAUG = [q; -1]
(memset row), KAUG = [k; |k|^2/2] (DVE square + cross-partition row
adds).  Ranking uses s = q.k - |k|^2/2 (= s_true/2 up to the constant
|q|^2 row term), computed with true-f32 matmul (f32r is ~1e-5 noisy and
misranks ~16% of rows).  8 PSUM chunks per tile are ACT-drained to a
[128,8192] f32 SBUF row; one DVE max8 + one max_index give the exact
top-8 columns per query, ties resolved to distinct positions lowest-
index-first, matching jax.lax.top_k.  max_index writes [128,8] into an
overlapping window of the persistent index tile so cols 3t..3t+3 keep
tile t's top-3; one [128,96] u16 DMA is the only output (24KB/core).

Host: one vectorized numpy pass over all 8 cores gathers the 3
neighbor coords/flows per query, recomputes exact f32 distances,
inverse-distance weights, warp, clip.  No re-rank, no rescue path.
Output placeholder buffers are uploaded once and reused across calls
(their content is never read by the device program).
"""

import numpy as np

B, C, N = 4, 3, 8192
NQ = 4096
NT = 32
EPS = 1e-10
CLAMP = 10.0

_CACHE = {}


def _build():
    if "nc" in _CACHE:
        return _CACHE["nc"]

    from contextlib import ExitStack
    from concourse import bacc, bass, tile
    from concourse import mybir

    nc = bacc.Bacc("TRN2", target_bir_lowering=False, debug=False,
                   enable_asserts=True, num_devices=1)
    f32 = mybir.dt.float32
    u16 = mybir.dt.uint16
    MULT = mybir.AluOpType.mult

    qr = nc.dram_tensor("qr", [3, NQ], f32, kind="ExternalInput").ap()
    kr = nc.dram_tensor("kr", [3, N], f32, kind="ExternalInput").ap()
    gidxo = nc.dram_tensor("gidxo", [128, 3 * NT], u16,
                           kind="ExternalOutput").ap()
    gvalo = nc.dram_tensor("gvalo", [128, 3 * NT], f32,
                           kind="ExternalOutput").ap()

    with tile.TileContext(nc) as tc, ExitStack() as ctx:
        cp = ctx.enter_context(tc.tile_pool(name="persist", bufs=1))
        tp = ctx.enter_context(tc.tile_pool(name="loop", bufs=2))
        pp = ctx.enter_context(tc.tile_pool(name="ps", bufs=4, space="PSUM"))

        # s = q.k - |k|^2/2 via one K=6 contraction:
        #   lhsT rows = [q0,q1,q2,-.5,-.5,-.5], rhs rows = [k0,k1,k2,k0^2,
        #   k1^2,k2^2].  Compute ops cannot start at partition>0, so the
        #   constant rows come from a full-tile memset overwritten by the q
        #   DMA, and the squared rows are squared at partition 0 then moved
        #   to partitions 3:6 by an SBUF->SBUF DMA (DMA may target any
        #   partition).
        Q6 = cp.tile([6, NQ], f32, tag="Q6", bufs=1, name="Q6")
        K6 = cp.tile([6, N], f32, tag="K6", bufs=1, name="K6")
        KSQ = cp.tile([3, N], f32, tag="KSQ", bufs=1, name="KSQ")

        nc.vector.memset(Q6[:, :], -0.5)
        nc.sync.dma_start(Q6[0:3, 0:128], qr[:, 0:128])
        nc.sync.dma_start(K6[0:3, 0:2730], kr[:, 0:2730])
        nc.scalar.dma_start(K6[0:3, 2730:5461], kr[:, 2730:5461])
        nc.gpsimd.dma_start(K6[0:3, 5461:8192], kr[:, 5461:8192])
        nc.sync.dma_start(Q6[0:3, 128:NQ], qr[:, 128:NQ])

        nc.vector.tensor_tensor(KSQ[:, :], K6[0:3, :], K6[0:3, :], MULT)
        nc.sync.dma_start(K6[3:6, :], KSQ[:, :])

        # cols 3t..3t+3 hold tile t's top-3; max/max_index write 8 cols per
        # tile, the 5 extra are overwritten by the next tile (engine-ordered
        # on DVE), with 5 pad cols for the last tile.
        GIDX3 = cp.tile([128, 3 * NT + 5], u16, tag="GIDX3", bufs=1,
                        name="GIDX3")
        GVAL3 = cp.tile([128, 3 * NT + 5], f32, tag="GVAL3", bufs=1,
                        name="GVAL3")

        for t in range(NT):
            lhsT = Q6[:, bass.ts(t, 128)]
            S = tp.tile([128, N], f32, tag="S", name="S")
            for k in range(8):
                P = pp.tile([128, 1024], f32, tag="P", bufs=4, name=f"P{k}")
                for i in range(2):
                    nc.tensor.matmul(
                        P[:, bass.ts(i, 512)], lhsT,
                        K6[:, 1024 * k + 512 * i:1024 * k + 512 * (i + 1)],
                        start=True, stop=True)
                nc.scalar.copy(S[:, 1024 * k:1024 * (k + 1)], P[:, :])
            TOP = GVAL3[:, 3 * t:3 * t + 8]
            nc.vector.max(TOP, S[:, :])
            nc.vector.max_index(GIDX3[:, 3 * t:3 * t + 8], TOP, S[:, :])

        nc.sync.dma_start(gidxo[:, :], GIDX3[:, 0:3 * NT])
        nc.sync.dma_start(gvalo[:, :], GVAL3[:, 0:3 * NT])

    nc.compile()
    _CACHE["nc"] = nc
    return nc


def _get_runner():
    if "runner" in _CACHE:
        return _CACHE["runner"]

    import jax
    from jax.sharding import Mesh, PartitionSpec, NamedSharding
    import warnings
    with warnings.catch_warnings():
        warnings.simplefilter("ignore")
        try:
            from jax.experimental.shard_map import shard_map
        except ImportError:
            from jax import shard_map
    from concourse import mybir
    from concourse.bass2jax import (
        install_neuronx_cc_hook,
        _bass_exec_p,
        partition_id_tensor,
    )

    nc = _build()
    n_cores = 8
    install_neuronx_cc_hook()
    partition_name = (nc.partition_id_tensor.name
                      if nc.partition_id_tensor else None)

    in_names, out_names, out_avals = [], [], []
    for alloc in nc.m.functions[0].allocations:
        if not isinstance(alloc, mybir.MemoryLocationSet):
            continue
        name = alloc.memorylocations[0].name
        if alloc.kind == "ExternalInput":
            if name != partition_name:
                in_names.append(name)
        elif alloc.kind == "ExternalOutput":
            out_names.append(name)
            shape = tuple(alloc.tensor_shape)
            dtype = mybir.dt.np(alloc.dtype)
            out_avals.append(jax.core.ShapedArray(shape, dtype))
    all_names = list(in_names) + list(out_names)
    if partition_name is not None:
        all_names.append(partition_name)

    def _body(*args):
        operands = list(args)
        if partition_name is not None:
            operands.append(partition_id_tensor())
        outs = _bass_exec_p.bind(
            *operands,
            out_avals=tuple(out_avals),
            in_names=tuple(all_names),
            out_names=tuple(out_names),
            lowering_input_output_aliases=(),
            sim_require_finite=True,
            sim_require_nnan=True,
            nc=nc,
        )
        return tuple(outs)

    devices = jax.devices()[:n_cores]
    mesh = Mesh(np.asarray(devices), ("core",))
    in_specs = (PartitionSpec("core"),) * (len(in_names) + len(out_names))
    out_specs = (PartitionSpec("core"),) * len(out_names)
    try:
        smapped = shard_map(_body, mesh=mesh, in_specs=in_specs,
                            out_specs=out_specs, check_vma=False)
    except TypeError:
        smapped = shard_map(_body, mesh=mesh, in_specs=in_specs,
                            out_specs=out_specs, check_rep=False)
    sharded = jax.jit(smapped)

    # output placeholder buffers: uploaded once, reused every call (the
    # device program never reads them)
    sh = NamedSharding(mesh, PartitionSpec("core"))
    resident_zeros = [
        jax.device_put(
            np.zeros((n_cores * a.shape[0], *a.shape[1:]), a.dtype), sh)
        for a in out_avals
    ]
    for z in resident_zeros:
        z.block_until_ready()

    runner = {
        "sharded": sharded,
        "in_names": in_names,
        "out_names": out_names,
        "out_shapes": [(tuple(a.shape), a.dtype) for a in out_avals],
        "zeros": resident_zeros,
        "n_cores": n_cores,
    }
    _CACHE["runner"] = runner
    return runner


def _run_device(concat_in):
    """Run the bass kernel on 8 cores; returns a collect() closure."""

    r = _get_runner()
    n_cores = r["n_cores"]
    out = r["sharded"](*concat_in, *r["zeros"])
    for a in out:
        for sh in a.addressable_shards:
            sh.data.copy_to_host_async()

    def collect():
        return [np.asarray(a) for a in out]

    return collect


def kernel(pos1, pos2, flow1):
    pos1 = np.asarray(pos1, dtype=np.float32)
    pos2 = np.asarray(pos2, dtype=np.float32)
    flow1 = np.asarray(flow1, dtype=np.float32)

    # core c = 2b + h handles queries pos2[b, :, h*NQ:(h+1)*NQ] against the
    # full batch-b database k = pos1[b] + flow1[b]
    k_all = pos1 + flow1                                     # [4, 3, 8192]
    qr_in = np.concatenate(
        [pos2[c // 2, :, (c % 2) * NQ:(c % 2 + 1) * NQ] for c in range(8)],
        axis=0)                                              # [24, 4096]
    kr_in = np.concatenate([k_all[c // 2] for c in range(8)], axis=0)

    collect = _run_device([qr_in, kr_in])

    # host-side array prep overlaps the device round trip
    q_all = pos2.transpose(0, 2, 1).reshape(8, NQ, 3)        # [8, 4096, 3]
    qq = np.einsum("qnj,qnj->qn", q_all, q_all)              # |q|^2 [8, 4096]
    flow_flat = flow1.transpose(0, 2, 1).reshape(B * N, 3)
    base = (np.arange(8, dtype=np.int32) // 2 * N)[:, None, None]

    gidx, gval = collect()                                   # [1024, 96] each
    # [p, 3t+j] per core -> query-major [8, 4096, 3]
    idx = gidx.reshape(8, 128, NT, 3).transpose(0, 2, 1, 3).reshape(8, NQ, 3)
    idx = np.minimum(idx.astype(np.int32), N - 1) + base     # flat into [32768]
    s3 = gval.reshape(8, 128, NT, 3).transpose(0, 2, 1, 3).reshape(8, NQ, 3)

    # s = q.k - |k|^2/2  ->  d2 = |q|^2 - 2s
    d2 = qq[:, :, None] - 2.0 * s3
    dist = np.maximum(np.sqrt(np.maximum(d2, 0.0)), EPS)
    inv = 1.0 / dist
    w = inv / inv.sum(-1, keepdims=True)                     # [8, 4096, 3]
    fg = flow_flat[idx]                                      # [8, 4096, 3, 3]
    flow2 = np.einsum("qnc,qncj->qnj", w, fg)                # [8, 4096, 3]
    res8 = np.clip(q_all - flow2, -CLAMP, CLAMP)             # [8, 4096, 3]
    # [8, 4096, 3] -> [4, 3, 8192] with n = h*NQ + pos
    return np.ascontiguousarray(
        res8.reshape(B, 2, NQ, 3).transpose(0, 3, 1, 2).reshape(B, C, N))
